# revision 42
# baseline (speedup 1.0000x reference)
"""Self-contained Trainium2 Bass kernel: causal self-attention with ALiBi bias.

Reference computation (B=2, T=2048, C=1024, H=16, Dh=64):
    qkv = x @ W_attn.T + b_attn; split into q,k,v heads
    att = softmax(q.k/sqrt(Dh) + slope_h*min(c-r,0), causal)
    y = (att @ v, heads concat) @ W_proj.T + b_proj

Sharding (8 cores): 2-way data parallel on batch x 4-way tensor parallel on
heads (4 heads/core). Each core computes qkv for its heads over its batch,
full TxT attention for those heads, and per-head-pair partial output
projections; the host sums the 8 partials per batch and adds b_proj.

v2 dataflow (fp16 on the matmul path, fp32 PSUM accumulate):
  - Host pre-transposes x (input "xT" [C,T] fp16), pre-scales W_q/b_q by
    1/sqrt(Dh), and provides the ALiBi aug rows, exp-bias columns and the
    causal mask-cap table, so the kernel does no PE transposes and no
    iota/act constant construction.
  - q.T/k.T [65,T] per head (row 64: q aug = -slope*(t%1024), k aug = ones)
    feed K=65 score matmuls emitting S.T[j,t] tiles; exp on ACT with
    per-partition bias slope*(j - tcp*1024) - M (M=2 guards fp16 overflow;
    all per-column/per-tile shifts cancel exactly in the normalization).
    Diagonal tiles are causal-masked with a DVE min against a 0/65504
    host table (min(exp,0)=0 also squashes masked-region fp16 overflow).
  - v computed in natural [t,d] orientation directly (moving operand = W_v),
    bias folded via a K=1 ones pre-matmul; assembled per (head, j-block)
    with a ones column so the PV matmul yields numerator and denominator.
  - P.T tiles (fp16 SBUF) feed PV directly; each block's PV is deferred a
    few blocks so the in-order PE queue never stalls behind exp; y.T is
    normalized via exact reciprocal + PE broadcast, with the broadcast+mul
    deferred two blocks past the reciprocal for the same reason.
  - Schedule: one continuous attention stream alternating tcp0/tcp1 heads
    (spreads ACT exp work across the whole timeline), with qkv build and
    the per-head-pair fp16 [C,T] projection partials interleaved into the
    PE slack; host sums the 8 partials per batch and adds b_proj.
"""

import math
import numpy as np

import concourse.bass as bass
import concourse.mybir as mybir
from concourse import bacc, tile
from concourse.bass_utils import run_bass_kernel_spmd

f32 = mybir.dt.float32
f32r = mybir.dt.float32r
f16 = mybir.dt.float16
AF = mybir.ActivationFunctionType
ALU = mybir.AluOpType

B, T, C, H, DH = 2, 2048, 1024, 16, 64
NCORES = 8
CPB = NCORES // B            # cores per batch (4)
HPC = H // CPB               # heads per core (4)
NHP = HPC // 2               # head pairs per core (2)
D_LOC = HPC * DH             # local feature dim (256)
HT = T // 2                  # half the sequence (1024)
NTC = T // 512               # 4 t-chunks
SLOPES = [2.0 ** (-8.0 / H * (h + 1)) for h in range(H)]
INV_SQRT_D = 1.0 / math.sqrt(DH)
M_MARGIN = 2.0               # exp-bias margin: keeps p <= e^(smax-M) in fp16
USE_APPROX_RECIP = False


def _bias_col(h, tcp, jb):
    # column index into the host-provided exp-bias table [128, 96]
    return h * 24 + (0 if tcp == 0 else 8) + jb


def build(nrep: int = 1, phases: str = 'full', **kw):
    nc = bacc.Bacc("TRN2", target_bir_lowering=False, debug=False)
    xT_d = nc.dram_tensor("xT", [C, T], f16, kind="ExternalInput")
    wqk_d = nc.dram_tensor("wqkT", [C, 2 * D_LOC], f16,
                           kind="ExternalInput")
    wv_d = nc.dram_tensor("wvT", [C, D_LOC], f16, kind="ExternalInput")
    bq_d = nc.dram_tensor("bq6", [128, 6], f32, kind="ExternalInput")
    bv_d = nc.dram_tensor("bvrow", [1, D_LOC], f16, kind="ExternalInput")
    wp_d = nc.dram_tensor("wpT", [D_LOC, C], f16, kind="ExternalInput")
    # aug rows live on partitions 0/32/64/96 (DVE partition offsets must be
    # 32-aligned)
    aug_d = nc.dram_tensor("aug", [97, HT], f16, kind="ExternalInput")
    onesr_d = nc.dram_tensor("onesrow", [1, HT], f16, kind="ExternalInput")
    bias_d = nc.dram_tensor("biascols", [128, 24 * HPC], f32,
                            kind="ExternalInput")
    mask_d = nc.dram_tensor("maskcap", [128, 4 * 512], f16,
                            kind="ExternalInput")
    out_d = [nc.dram_tensor(f"out{p}", [C, T], f16, kind="ExternalOutput")
             for p in range(NHP)]

    with tile.TileContext(nc) as tc:
        with tc.tile_pool(name="const", bufs=1) as cp:
            ones16 = cp.tile([1, 128], f16)
            nc.vector.memset(ones16[:], 1.0)
            ones_f = cp.tile([1, 128], f32)
            nc.vector.memset(ones_f[:], 1.0)
            ones_r32 = cp.tile([1, 128], f32r)
            nc.vector.tensor_copy(ones_r32[:], ones_f[:])
            ones32c = cp.tile([128, 32], f16)
            nc.vector.memset(ones32c[:], 1.0)
            bias_sb = cp.tile([128, 24 * HPC], f32)
            nc.gpsimd.dma_start(bias_sb[:], bias_d.ap()[:, :])

            def body(_iv=None):
                with tc.tile_pool(name="long", bufs=1) as lp:
                    xt = [lp.tile([128, T], f16, name=f"xt{cc}",
                                  tag=f"xt{cc}") for cc in range(8)]
                    qT = [[lp.tile([65, HT], f16, name=f"qT{th}_{h}",
                                   tag=f"qT{th}_{h}") for h in range(HPC)]
                          for th in range(2)]
                    kT = [[lp.tile([65, HT], f16, name=f"kT{th}_{h}",
                                   tag=f"kT{th}_{h}") for h in range(HPC)]
                          for th in range(2)]
                    # vp[th]: [128 keys, (h,jb,c) = 4*8*65]; c=64 is ones
                    vp = [lp.tile([128, HPC * 8 * 65], f16, name=f"vp{th}",
                                  tag=f"vp{th}") for th in range(2)]
                    yT = [[lp.tile([128, HT], f16, name=f"yT{tcp}_{hp}",
                                   tag=f"yT{tcp}_{hp}") for hp in range(NHP)]
                          for tcp in range(2)]
                    w_sb = [lp.tile([128, 2 * D_LOC], f16, name=f"wsb{cc}",
                                    tag=f"wsb{cc}") for cc in range(8)]
                    wv_sb = [lp.tile([128, D_LOC], f16, name=f"wvsb{cc}",
                                     tag=f"wvsb{cc}") for cc in range(8)]
                    wpt = [lp.tile([128, C], f16, name=f"wp{hp}",
                                   tag=f"wp{hp}") for hp in range(NHP)]
                    bq_sb = lp.tile([128, 6], f32, name="bq", tag="bq")
                    bv_sb = lp.tile([1, D_LOC], f16, name="bv", tag="bv")
                    aug_sb = lp.tile([97, HT], f16, name="aug", tag="aug")
                    mask_sb = lp.tile([128, 4 * 512], f16, name="mask",
                                      tag="mask")
                    ones_sb = lp.tile([1, HT], f16, name="onesr",
                                      tag="onesr")

                    for cc in range(8):
                        nc.scalar.dma_start(
                            w_sb[cc][:],
                            wqk_d.ap()[cc * 128:(cc + 1) * 128, :])
                    nc.scalar.dma_start(bq_sb[:], bq_d.ap()[:, :])
                    nc.scalar.dma_start(bv_sb[:], bv_d.ap()[:, :])
                    for cc in range(8):
                        nc.scalar.dma_start(
                            wv_sb[cc][:],
                            wv_d.ap()[cc * 128:(cc + 1) * 128, :])
                    nc.gpsimd.dma_start(aug_sb[:], aug_d.ap()[:, :])
                    nc.gpsimd.dma_start(ones_sb[:], onesr_d.ap()[:, :])
                    nc.gpsimd.dma_start(mask_sb[:], mask_d.ap()[:, :])
                    # aug rows + ones rows + vp ones columns
                    for th in range(2):
                        for h in range(HPC):
                            nc.vector.tensor_copy(
                                qT[th][h][64:65, :],
                                aug_sb[32 * h:32 * h + 1, :])
                            nc.vector.tensor_copy(
                                kT[th][h][64:65, :], ones_sb[:])
                        vp4 = vp[th].rearrange("p (g c) -> p g c", c=65)
                        nc.vector.tensor_copy(
                            vp4[:, :, 64:65],
                            ones32c[:].rearrange("p (g o) -> p g o", o=1))

                    def emit_xdma(th):
                        # quarter granularity: first qk group waits on 2MB,
                        # not the full half
                        for tc4 in (2 * th, 2 * th + 1):
                            for cc in range(8):
                                nc.sync.dma_start(
                                    xt[cc][:, tc4 * 512:(tc4 + 1) * 512],
                                    xT_d.ap()[cc * 128:(cc + 1) * 128,
                                              tc4 * 512:(tc4 + 1) * 512])

                    def emit_qk(th, hp, bld):
                        for tcl in range(2):
                            tc4 = th * 2 + tcl
                            for typ in range(2):        # q, k
                                ps = bld.tile([128, 512], f32, tag="bld")
                                for cc in range(8):
                                    nc.tensor.matmul(
                                        ps[:],
                                        w_sb[cc][:, typ * D_LOC + hp * 128:
                                                 typ * D_LOC + (hp + 1) * 128],
                                        xt[cc][:, tc4 * 512:(tc4 + 1) * 512],
                                        start=(cc == 0), stop=(cc == 7))
                                dst = qT[th] if typ == 0 else kT[th]
                                for sub in range(2):
                                    h = 2 * hp + sub
                                    nc.vector.tensor_scalar_add(
                                        dst[h][0:64, tcl * 512:
                                               (tcl + 1) * 512],
                                        ps[sub * 64:(sub + 1) * 64, :],
                                        bq_sb[sub * 64:(sub + 1) * 64,
                                              typ * 2 + hp:typ * 2 + hp + 1])
                                yield

                    def emit_v(th, bld):
                        for tbl in range(8):            # v, natural [t, d]
                            tb = th * 8 + tbl
                            jbl = tb % 8
                            psv = bld.tile([128, 256], f32, tag="bld")
                            nc.tensor.matmul(psv[:], ones16[:], bv_sb[:],
                                             start=True, stop=False)
                            for cc in range(8):
                                nc.tensor.matmul(
                                    psv[:],
                                    xt[cc][:, tb * 128:(tb + 1) * 128],
                                    wv_sb[cc][:],
                                    start=False, stop=(cc == 7))
                            vp5 = vp[th].rearrange(
                                "p (h j c) -> p h j c", h=HPC, c=65)
                            nc.vector.tensor_copy(
                                vp5[:, :, jbl, 0:64],
                                psv[:].rearrange("p (h d) -> p h d", d=64))
                            yield

                    bcpool = {}   # holder: bc tiles live outside the s pool

                    def emit_attn(pairs, spool, ypool, deferred):
                        def tick():
                            for e in deferred:
                                e[0] -= 1
                            while deferred and deferred[0][0] <= 0:
                                deferred.pop(0)[1]()

                        for tcp, h in pairs:
                            hp, sub = divmod(h, 2)
                            y_acc = {}
                            for tcl in range(2):
                                y_acc[tcl] = ypool.tile([65, 512], f32,
                                                        name="y_acc", tag="y")
                            def norm_recip(tcp, tcl, y_acc, hp, sub):
                                rec_r = nrmp.tile([1, 512], f32r, name="recr",
                                                  tag="recr")
                                with nc.allow_low_precision(
                                        reason="softmax denominator bcast"):
                                    nc.vector.reciprocal(
                                        rec_r[:], y_acc[tcl][64:65, :])

                                def apply():
                                    bcst = bcpool["p"].tile(
                                        [128, 512], f32, name="bc",
                                        tag=bcpool["tag"])
                                    nc.tensor.matmul(bcst[:], ones_r32[:],
                                                     rec_r[:], start=True,
                                                     stop=True)
                                    bcs = nrmp.tile([128, 512], f16,
                                                    name="bcs", tag="bcs")
                                    nc.vector.tensor_copy(bcs[:], bcst[:])
                                    nc.vector.tensor_mul(
                                        yT[tcp][hp][sub * 64:(sub + 1) * 64,
                                                    tcl * 512:
                                                    (tcl + 1) * 512],
                                        y_acc[tcl][0:64, :], bcs[0:64, :])
                                return apply

                            pending = []

                            def emit_pv(jb, tc4, pt, lead):
                                jbl = jb % 8
                                nc.tensor.matmul(
                                    y_acc[tc4 - 2 * tcp][:, lead:512],
                                    vp[jb // 8][:, (h * 8 + jbl) * 65:
                                                (h * 8 + jbl) * 65 + 65],
                                    pt[:, lead:512],
                                    start=(jb == 0),
                                    stop=(jb == 4 * tc4 + 3))

                            for jb in range(8 * (tcp + 1)):
                                tc_lo = max(2 * tcp, jb // 4)
                                o = jb * 128 - tcp * HT
                                kTt = kT[jb // 8][h]
                                jbl = jb % 8
                                for tc4 in range(tc_lo, 2 * tcp + 2):
                                    glo = (tc4 - 2 * tcp) * 512
                                    # leading fully-masked columns of a
                                    # diagonal tile: skip them everywhere
                                    lead = max(0, o - glo)
                                    s = spool.tile([128, 512], f32, name="s",
                                                   tag="s")
                                    nc.tensor.matmul(
                                        s[:, lead:512],
                                        kTt[:, jbl * 128:(jbl + 1) * 128],
                                        qT[tc4 // 2][h][:, (tc4 % 2) * 512
                                                        + lead:
                                                        (tc4 % 2 + 1) * 512],
                                        start=True, stop=True)
                                    pt = ptp.tile([128, 512], f16, name="pt",
                                                  tag="pt")
                                    bc = _bias_col(h, tcp, jb)
                                    nc.scalar.activation(
                                        pt[:, lead:512], s[:, lead:512],
                                        AF.Exp,
                                        bias=bias_sb[:, bc:bc + 1], scale=1.0)
                                    if o + 128 > glo:
                                        k = lead // 128
                                        w = min(o + 128, glo + 512) - glo
                                        nc.vector.tensor_tensor(
                                            pt[:, lead:w], pt[:, lead:w],
                                            mask_sb[:, k * 512 + lead:
                                                    k * 512 + w],
                                            ALU.min)
                                    # defer PV one block so the next S is in
                                    # the PE queue before PV blocks on exp
                                    pending.append((jb, tc4, pt, lead))
                                    if len(pending) > 3:
                                        emit_pv(*pending.pop(0))
                                    yield
                                    tick()
                                if jb == 4 * (2 * tcp) + 3:
                                    while pending:
                                        emit_pv(*pending.pop(0))
                                    deferred.append(
                                        [2, norm_recip(tcp, 0, y_acc,
                                                       hp, sub)])
                                    yield
                                    tick()
                            while pending:
                                emit_pv(*pending.pop(0))
                            deferred.append(
                                [2, norm_recip(tcp, 1, y_acc, hp, sub)])
                            yield
                            tick()

                    def emit_proj(tcp, hp, ypool, evac='dve', tag="pp"):
                        for cc in range(8):
                            for tcl in range(2):
                                tc4 = 2 * tcp + tcl
                                ps = ypool.tile([128, 512], f32, name="pp",
                                                tag=tag)
                                nc.tensor.matmul(
                                    ps[:],
                                    wpt[hp][:, cc * 128:(cc + 1) * 128],
                                    yT[tcp][hp][:, tcl * 512:(tcl + 1) * 512],
                                    start=True, stop=True)
                                ob = obp.tile([128, 512], f16, name="ob",
                                              tag="ob")
                                if evac == 'act':
                                    nc.scalar.copy(ob[:], ps[:])
                                elif evac == 'mix' and cc % 2 == 0:
                                    nc.scalar.copy(ob[:], ps[:])
                                else:
                                    nc.vector.tensor_copy(ob[:], ps[:])
                                nc.sync.dma_start(
                                    out_d[hp].ap()[cc * 128:(cc + 1) * 128,
                                                   tc4 * 512:(tc4 + 1) * 512],
                                    ob[:])
                                yield

                    def drain(g):
                        for _ in g:
                            pass

                    def chain(*gens):
                        for g in gens:
                            yield from g

                    def interleave(ga, gb, ratio, drain_b=True):
                        alive_a = alive_b = True
                        while alive_a or (alive_b and drain_b):
                            for _ in range(ratio):
                                if alive_a:
                                    try:
                                        next(ga)
                                    except StopIteration:
                                        alive_a = False
                            if alive_b:
                                try:
                                    next(gb)
                                except StopIteration:
                                    alive_b = False

                    def pad(n):
                        for _ in range(n):
                            yield

                    def merge2(ga, gb):
                        alive_a = alive_b = True
                        while alive_a or alive_b:
                            if alive_a:
                                try:
                                    next(ga)
                                except StopIteration:
                                    alive_a = False
                            if alive_b:
                                try:
                                    next(gb)
                                except StopIteration:
                                    alive_b = False
                            yield

                    with tc.tile_pool(name="pt", bufs=8) as ptp, \
                         tc.tile_pool(name="nrm", bufs=4) as nrmp, \
                         tc.tile_pool(name="ob", bufs=3) as obp, \
                         tc.tile_pool(name="s", bufs=2, space="PSUM") as sp, \
                         tc.tile_pool(name="y", bufs=4, space="PSUM") as yp:
                        deferred = []   # [countdown, norm-apply fn]
                        with tc.tile_pool(name="bld", bufs=2,
                                          space="PSUM") as bld:
                            emit_xdma(0)
                            drain(emit_qk(0, 0, bld))
                            emit_xdma(1)
                            for hp in range(NHP):
                                nc.gpsimd.dma_start(
                                    wpt[hp][:],
                                    wp_d.ap()[hp * 128:(hp + 1) * 128, :])
                            if phases == 'build':
                                drain(emit_v(0, bld))
                                drain(emit_qk(0, 1, bld))
                                drain(emit_qk(1, 0, bld))
                                drain(emit_v(1, bld))
                                drain(emit_qk(1, 1, bld))
                            else:
                                bcpool["p"] = bld
                                bcpool["tag"] = "bld"
                                interleave(
                                    emit_attn([(0, 0)], sp, yp, deferred),
                                    chain(emit_v(0, bld),
                                          emit_qk(1, 0, bld)), 1)
                                interleave(
                                    merge2(
                                        emit_attn([(1, 0)], sp, yp,
                                                  deferred),
                                        emit_attn([(0, 1)], sp, yp,
                                                  deferred)),
                                    chain(emit_v(1, bld),
                                          emit_qk(0, 1, bld),
                                          emit_qk(1, 1, bld)), 1)
                        if phases != 'build':
                            with tc.tile_pool(name="pp", bufs=2,
                                              space="PSUM") as ppp:
                                bcpool["p"] = ppp
                                bcpool["tag"] = "pp"
                                bg = chain(pad(6),
                                           emit_proj(0, 0, ppp),
                                           pad(10),
                                           emit_proj(1, 0, ppp),
                                           pad(2),
                                           emit_proj(0, 1, ppp))
                                interleave(
                                    merge2(
                                        emit_attn([(1, 1)], sp, yp,
                                                  deferred),
                                        emit_attn([(0, 2)], sp, yp,
                                                  deferred)),
                                    bg, 1, drain_b=False)
                                interleave(
                                    merge2(
                                        emit_attn([(1, 2)], sp, yp,
                                                  deferred),
                                        emit_attn([(0, 3)], sp, yp,
                                                  deferred)),
                                    bg, 1, drain_b=False)
                                interleave(
                                    emit_attn([(1, 3)], sp, yp, deferred),
                                    bg, 2, drain_b=False)
                                while deferred:
                                    deferred.pop(0)[1]()
                                interleave(
                                    emit_proj(1, 1, yp, tag="y"),
                                    bg, 1)

            if phases == 'build':
                                drain(emit_v(0, bld))
                                drain(emit_qk(0, 1, bld))
                                drain(emit_qk(1, 0, bld))
                                drain(emit_v(1, bld))
                                drain(emit_qk(1, 1, bld))
                            else:
                                def build_rest1():
                                    yield from emit_v(0, bld)
                                    yield from emit_qk(1, 0, bld)
                                interleave(
                                    emit_attn([(0, 0)], sp, yp, deferred),
                                    build_rest1(), 1)
                                interleave(
                                    emit_attn([(1, 0)], sp, yp, deferred),
                                    emit_v(1, bld), 3)
                                def build_rest2():
                                    yield from emit_qk(0, 1, bld)
                                    yield from emit_qk(1, 1, bld)
                                interleave(
                                    emit_attn([(0, 1), (1, 1)], sp, yp,
                                              deferred),
                                    build_rest2(), 4)
                        if phases != 'build':
                            with tc.tile_pool(name="pp", bufs=2,
                                              space="PSUM") as ppp:
                                bg = chain(emit_proj(0, 0, ppp, 'mix'),
                                           emit_proj(1, 0, ppp, 'mix'),
                                           pad(6),
                                           emit_proj(0, 1, ppp, 'mix'))
                                interleave(
                                    emit_attn([(0, 2), (1, 2)], sp, yp,
                                              deferred),
                                    bg, 2, drain_b=False)
                                interleave(
                                    emit_attn([(0, 3), (1, 3)], sp, yp,
                                              deferred),
                                    bg, 2, drain_b=False)
                                while deferred:
                                    deferred.pop(0)[1]()
                                interleave(
                                    emit_proj(1, 1, yp, tag="y"),
                                    bg, 1)

            if phases == 'build':
            if phases == 'build':
                zt = cp.tile([128, 512], f16, tag="zt")
                nc.vector.memset(zt[:], 0.0)
                for p in range(NHP):
                    for cc in range(8):
                        for tc4 in range(NTC):
                            nc.sync.dma_start(
                                out_d[p].ap()[cc * 128:(cc + 1) * 128,
                                              tc4 * 512:(tc4 + 1) * 512],
                                zt[:])
            if nrep > 1:
                with tc.For_i(0, nrep, 1):
                    body()
            else:
                body()
    nc.compile()
    return nc


def shard_inputs(x, W_attn, b_attn, W_proj, b_proj):
    x = np.asarray(x, np.float32)
    W_attn = np.asarray(W_attn, np.float32)
    b_attn = np.asarray(b_attn, np.float32)
    W_proj = np.asarray(W_proj, np.float32)
    in_maps = []
    for core in range(NCORES):
        b = core // CPB
        hg = core % CPB
        heads = list(range(hg * HPC, (hg + 1) * HPC))
        rows, brows = [], []
        for typ in range(3):
            for h in heads:
                r0 = typ * C + h * DH
                w = W_attn[r0:r0 + DH]
                bb = b_attn[r0:r0 + DH]
                if typ == 0:
                    w = w * INV_SQRT_D
                    bb = bb * INV_SQRT_D
                rows.append(w)
                brows.append(bb)
        wfull = np.ascontiguousarray(np.concatenate(rows, 0).T)  # [C, 768]
        wqkT = np.ascontiguousarray(wfull[:, :2 * D_LOC]).astype(np.float16)
        wvT = np.ascontiguousarray(wfull[:, 2 * D_LOC:]).astype(np.float16)
        bqkv = np.concatenate(brows).astype(np.float32)   # [768]
        # bq6 [128, 6]: col typ*2+hp = bias rows for that (typ, head-pair)
        bq6 = np.zeros((128, 6), np.float32)
        for typ in range(2):
            for hp in range(NHP):
                bq6[:, typ * 2 + hp] = bqkv[typ * D_LOC + hp * 128:
                                            typ * D_LOC + (hp + 1) * 128]
        bvrow = bqkv[2 * D_LOC:3 * D_LOC][None, :].astype(np.float16)
        cols = np.concatenate([np.arange(h * DH, (h + 1) * DH) for h in heads])
        wpT = np.ascontiguousarray(W_proj[:, cols].T).astype(np.float16)
        tt = np.arange(HT, dtype=np.float32)
        aug = np.zeros((97, HT), np.float16)
        for hl, h in enumerate(heads):
            aug[32 * hl] = (-SLOPES[h] * tt).astype(np.float16)
        # exp-bias table [128, 24*HPC]: slope*(jb*128 + jp - tcp*1024) - M
        jp = np.arange(128, dtype=np.float32)
        bias = np.zeros((128, 24 * HPC), np.float32)
        for hl, h in enumerate(heads):
            for tcp in range(2):
                for jb in range(8 * (tcp + 1)):
                    bias[:, _bias_col(hl, tcp, jb)] = (
                        SLOPES[h] * (jb * 128 + jp - tcp * HT) - M_MARGIN)
        tt512 = np.arange(512)[None, :]
        jl = np.arange(128)[:, None]
        maskcap = np.zeros((128, 4 * 512), np.float16)
        for k in range(4):
            maskcap[:, k * 512:(k + 1) * 512] = np.where(
                tt512 >= k * 128 + jl, np.float16(65504.0), np.float16(0.0))
        in_maps.append({
            "xT": np.ascontiguousarray(x[b].T).astype(np.float16),
            "wqkT": wqkT, "wvT": wvT, "bq6": bq6, "bvrow": bvrow, "wpT": wpT,
            "aug": aug, "biascols": bias, "maskcap": maskcap,
            "onesrow": np.ones((1, HT), np.float16),
        })
    return in_maps


def unshard(results, b_proj):
    y = np.zeros((B, T, C), np.float32)
    for core in range(NCORES):
        b = core // CPB
        for p in range(NHP):
            y[b] += results[core][f"out{p}"].T.astype(np.float32)
    y += np.asarray(b_proj, np.float32)[None, None, :]
    return y


_BUILD_CACHE = {}


def _built(nrep: int = 1):
    if nrep not in _BUILD_CACHE:
        _BUILD_CACHE[nrep] = build(nrep)
    return _BUILD_CACHE[nrep]


def kernel(**inputs) -> np.ndarray:
    in_maps = shard_inputs(inputs["x"], inputs["W_attn"], inputs["b_attn"],
                           inputs["W_proj"], inputs["b_proj"])
    nc = _built(1)
    res = run_bass_kernel_spmd(nc, in_maps, core_ids=list(range(NCORES)))
    return unshard(res.results, inputs["b_proj"])


# revision 43
# speedup vs baseline: 1.0447x; 1.0447x over previous
"""Self-contained Trainium2 Bass kernel: causal self-attention with ALiBi bias.

Reference computation (B=2, T=2048, C=1024, H=16, Dh=64):
    qkv = x @ W_attn.T + b_attn; split into q,k,v heads
    att = softmax(q.k/sqrt(Dh) + slope_h*min(c-r,0), causal)
    y = (att @ v, heads concat) @ W_proj.T + b_proj

Sharding (8 cores): 2-way data parallel on batch x 4-way tensor parallel on
heads (4 heads/core). Each core computes qkv for its heads over its batch,
full TxT attention for those heads, and per-head-pair partial output
projections; the host sums the 8 partials per batch and adds b_proj.

v2 dataflow (fp16 on the matmul path, fp32 PSUM accumulate):
  - Host pre-transposes x (input "xT" [C,T] fp16), pre-scales W_q/b_q by
    1/sqrt(Dh), and provides the ALiBi aug rows, exp-bias columns and the
    causal mask-cap table, so the kernel does no PE transposes and no
    iota/act constant construction.
  - q.T/k.T [65,T] per head (row 64: q aug = -slope*(t%1024), k aug = ones)
    feed K=65 score matmuls emitting S.T[j,t] tiles; exp on ACT with
    per-partition bias slope*(j - tcp*1024) - M (M=2 guards fp16 overflow;
    all per-column/per-tile shifts cancel exactly in the normalization).
    Diagonal tiles are causal-masked with a DVE min against a 0/65504
    host table (min(exp,0)=0 also squashes masked-region fp16 overflow).
  - v computed in natural [t,d] orientation directly (moving operand = W_v),
    bias folded via a K=1 ones pre-matmul; assembled per (head, j-block)
    with a ones column so the PV matmul yields numerator and denominator.
  - P.T tiles (fp16 SBUF) feed PV directly; each block's PV is deferred a
    few blocks so the in-order PE queue never stalls behind exp; y.T is
    normalized via exact reciprocal + PE broadcast, with the broadcast+mul
    deferred two blocks past the reciprocal for the same reason.
  - Schedule: one continuous attention stream alternating tcp0/tcp1 heads
    (spreads ACT exp work across the whole timeline), with qkv build and
    the per-head-pair fp16 [C,T] projection partials interleaved into the
    PE slack; host sums the 8 partials per batch and adds b_proj.
"""

import math
import numpy as np

import concourse.bass as bass
import concourse.mybir as mybir
from concourse import bacc, tile
from concourse.bass_utils import run_bass_kernel_spmd

f32 = mybir.dt.float32
f32r = mybir.dt.float32r
f16 = mybir.dt.float16
AF = mybir.ActivationFunctionType
ALU = mybir.AluOpType

B, T, C, H, DH = 2, 2048, 1024, 16, 64
NCORES = 8
CPB = NCORES // B            # cores per batch (4)
HPC = H // CPB               # heads per core (4)
NHP = HPC // 2               # head pairs per core (2)
D_LOC = HPC * DH             # local feature dim (256)
HT = T // 2                  # half the sequence (1024)
NTC = T // 512               # 4 t-chunks
SLOPES = [2.0 ** (-8.0 / H * (h + 1)) for h in range(H)]
INV_SQRT_D = 1.0 / math.sqrt(DH)
M_MARGIN = 2.0               # exp-bias margin: keeps p <= e^(smax-M) in fp16
USE_APPROX_RECIP = False


def _bias_col(h, tcp, jb):
    # column index into the host-provided exp-bias table [128, 96]
    return h * 24 + (0 if tcp == 0 else 8) + jb


def build(nrep: int = 1, phases: str = 'full', **kw):
    nc = bacc.Bacc("TRN2", target_bir_lowering=False, debug=False)
    xT_d = nc.dram_tensor("xT", [C, T], f16, kind="ExternalInput")
    wqk_d = nc.dram_tensor("wqkT", [C, 2 * D_LOC], f16,
                           kind="ExternalInput")
    wv_d = nc.dram_tensor("wvT", [C, D_LOC], f16, kind="ExternalInput")
    bq_d = nc.dram_tensor("bq6", [128, 6], f32, kind="ExternalInput")
    bv_d = nc.dram_tensor("bvrow", [1, D_LOC], f16, kind="ExternalInput")
    wp_d = nc.dram_tensor("wpT", [D_LOC, C], f16, kind="ExternalInput")
    # aug rows live on partitions 0/32/64/96 (DVE partition offsets must be
    # 32-aligned)
    aug_d = nc.dram_tensor("aug", [97, HT], f16, kind="ExternalInput")
    onesr_d = nc.dram_tensor("onesrow", [1, HT], f16, kind="ExternalInput")
    bias_d = nc.dram_tensor("biascols", [128, 24 * HPC], f32,
                            kind="ExternalInput")
    mask_d = nc.dram_tensor("maskcap", [128, 4 * 512], f16,
                            kind="ExternalInput")
    out_d = [nc.dram_tensor(f"out{p}", [C, T], f16, kind="ExternalOutput")
             for p in range(NHP)]

    with tile.TileContext(nc) as tc:
        with tc.tile_pool(name="const", bufs=1) as cp:
            ones16 = cp.tile([1, 128], f16)
            nc.vector.memset(ones16[:], 1.0)
            ones_f = cp.tile([1, 128], f32)
            nc.vector.memset(ones_f[:], 1.0)
            ones_r32 = cp.tile([1, 128], f32r)
            nc.vector.tensor_copy(ones_r32[:], ones_f[:])
            ones32c = cp.tile([128, 32], f16)
            nc.vector.memset(ones32c[:], 1.0)
            bias_sb = cp.tile([128, 24 * HPC], f32)
            nc.gpsimd.dma_start(bias_sb[:], bias_d.ap()[:, :])

            def body(_iv=None):
                with tc.tile_pool(name="long", bufs=1) as lp:
                    xt = [lp.tile([128, T], f16, name=f"xt{cc}",
                                  tag=f"xt{cc}") for cc in range(8)]
                    qT = [[lp.tile([65, HT], f16, name=f"qT{th}_{h}",
                                   tag=f"qT{th}_{h}") for h in range(HPC)]
                          for th in range(2)]
                    kT = [[lp.tile([65, HT], f16, name=f"kT{th}_{h}",
                                   tag=f"kT{th}_{h}") for h in range(HPC)]
                          for th in range(2)]
                    # vp[th]: [128 keys, (h,jb,c) = 4*8*65]; c=64 is ones
                    vp = [lp.tile([128, HPC * 8 * 65], f16, name=f"vp{th}",
                                  tag=f"vp{th}") for th in range(2)]
                    yT = [[lp.tile([128, HT], f16, name=f"yT{tcp}_{hp}",
                                   tag=f"yT{tcp}_{hp}") for hp in range(NHP)]
                          for tcp in range(2)]
                    w_sb = [lp.tile([128, 2 * D_LOC], f16, name=f"wsb{cc}",
                                    tag=f"wsb{cc}") for cc in range(8)]
                    wv_sb = [lp.tile([128, D_LOC], f16, name=f"wvsb{cc}",
                                     tag=f"wvsb{cc}") for cc in range(8)]
                    wpt = [lp.tile([128, C], f16, name=f"wp{hp}",
                                   tag=f"wp{hp}") for hp in range(NHP)]
                    bq_sb = lp.tile([128, 6], f32, name="bq", tag="bq")
                    bv_sb = lp.tile([1, D_LOC], f16, name="bv", tag="bv")
                    aug_sb = lp.tile([97, HT], f16, name="aug", tag="aug")
                    mask_sb = lp.tile([128, 4 * 512], f16, name="mask",
                                      tag="mask")
                    ones_sb = lp.tile([1, HT], f16, name="onesr",
                                      tag="onesr")

                    for cc in range(8):
                        nc.scalar.dma_start(
                            w_sb[cc][:],
                            wqk_d.ap()[cc * 128:(cc + 1) * 128, :])
                    nc.scalar.dma_start(bq_sb[:], bq_d.ap()[:, :])
                    nc.scalar.dma_start(bv_sb[:], bv_d.ap()[:, :])
                    for cc in range(8):
                        nc.scalar.dma_start(
                            wv_sb[cc][:],
                            wv_d.ap()[cc * 128:(cc + 1) * 128, :])
                    nc.gpsimd.dma_start(aug_sb[:], aug_d.ap()[:, :])
                    nc.gpsimd.dma_start(ones_sb[:], onesr_d.ap()[:, :])
                    nc.gpsimd.dma_start(mask_sb[:], mask_d.ap()[:, :])
                    # aug rows + ones rows + vp ones columns
                    for th in range(2):
                        for h in range(HPC):
                            nc.vector.tensor_copy(
                                qT[th][h][64:65, :],
                                aug_sb[32 * h:32 * h + 1, :])
                            nc.vector.tensor_copy(
                                kT[th][h][64:65, :], ones_sb[:])
                        vp4 = vp[th].rearrange("p (g c) -> p g c", c=65)
                        nc.vector.tensor_copy(
                            vp4[:, :, 64:65],
                            ones32c[:].rearrange("p (g o) -> p g o", o=1))

                    def emit_xdma(th):
                        for cc in range(8):
                            nc.sync.dma_start(
                                xt[cc][:, th * HT:(th + 1) * HT],
                                xT_d.ap()[cc * 128:(cc + 1) * 128,
                                          th * HT:(th + 1) * HT])

                    def emit_qk(th, hp, bld):
                        for tcl in range(2):
                            tc4 = th * 2 + tcl
                            for typ in range(2):        # q, k
                                ps = bld.tile([128, 512], f32, tag="bld")
                                for cc in range(8):
                                    nc.tensor.matmul(
                                        ps[:],
                                        w_sb[cc][:, typ * D_LOC + hp * 128:
                                                 typ * D_LOC + (hp + 1) * 128],
                                        xt[cc][:, tc4 * 512:(tc4 + 1) * 512],
                                        start=(cc == 0), stop=(cc == 7))
                                dst = qT[th] if typ == 0 else kT[th]
                                for sub in range(2):
                                    h = 2 * hp + sub
                                    nc.vector.tensor_scalar_add(
                                        dst[h][0:64, tcl * 512:
                                               (tcl + 1) * 512],
                                        ps[sub * 64:(sub + 1) * 64, :],
                                        bq_sb[sub * 64:(sub + 1) * 64,
                                              typ * 2 + hp:typ * 2 + hp + 1])
                                yield

                    def emit_v(th, bld):
                        for tbl in range(8):            # v, natural [t, d]
                            tb = th * 8 + tbl
                            jbl = tb % 8
                            psv = bld.tile([128, 256], f32, tag="bld")
                            nc.tensor.matmul(psv[:], ones16[:], bv_sb[:],
                                             start=True, stop=False)
                            for cc in range(8):
                                nc.tensor.matmul(
                                    psv[:],
                                    xt[cc][:, tb * 128:(tb + 1) * 128],
                                    wv_sb[cc][:],
                                    start=False, stop=(cc == 7))
                            vp5 = vp[th].rearrange(
                                "p (h j c) -> p h j c", h=HPC, c=65)
                            nc.vector.tensor_copy(
                                vp5[:, :, jbl, 0:64],
                                psv[:].rearrange("p (h d) -> p h d", d=64))
                            yield

                    bcpool = {}   # holder: bc tiles live outside the s pool

                    def emit_attn(pairs, spool, ypool, deferred):
                        def tick():
                            for e in deferred:
                                e[0] -= 1
                            while deferred and deferred[0][0] <= 0:
                                deferred.pop(0)[1]()

                        for tcp, h in pairs:
                            hp, sub = divmod(h, 2)
                            y_acc = {}
                            for tcl in range(2):
                                y_acc[tcl] = ypool.tile([65, 512], f32,
                                                        name="y_acc", tag="y")
                            def norm_recip(tcp, tcl, y_acc, hp, sub):
                                rec_r = nrmp.tile([1, 512], f32r, name="recr",
                                                  tag="recr")
                                with nc.allow_low_precision(
                                        reason="softmax denominator bcast"):
                                    nc.vector.reciprocal(
                                        rec_r[:], y_acc[tcl][64:65, :])

                                def apply():
                                    bcst = bcpool["p"].tile(
                                        [128, 512], f32, name="bc",
                                        tag=bcpool["tag"])
                                    nc.tensor.matmul(bcst[:], ones_r32[:],
                                                     rec_r[:], start=True,
                                                     stop=True)
                                    bcs = nrmp.tile([128, 512], f16,
                                                    name="bcs", tag="bcs")
                                    nc.vector.tensor_copy(bcs[:], bcst[:])
                                    nc.vector.tensor_mul(
                                        yT[tcp][hp][sub * 64:(sub + 1) * 64,
                                                    tcl * 512:
                                                    (tcl + 1) * 512],
                                        y_acc[tcl][0:64, :], bcs[0:64, :])
                                return apply

                            pending = []

                            def emit_pv(jb, tc4, pt, lead):
                                jbl = jb % 8
                                nc.tensor.matmul(
                                    y_acc[tc4 - 2 * tcp][:, lead:512],
                                    vp[jb // 8][:, (h * 8 + jbl) * 65:
                                                (h * 8 + jbl) * 65 + 65],
                                    pt[:, lead:512],
                                    start=(jb == 0),
                                    stop=(jb == 4 * tc4 + 3))

                            for jb in range(8 * (tcp + 1)):
                                tc_lo = max(2 * tcp, jb // 4)
                                o = jb * 128 - tcp * HT
                                kTt = kT[jb // 8][h]
                                jbl = jb % 8
                                for tc4 in range(tc_lo, 2 * tcp + 2):
                                    glo = (tc4 - 2 * tcp) * 512
                                    # leading fully-masked columns of a
                                    # diagonal tile: skip them everywhere
                                    lead = max(0, o - glo)
                                    s = spool.tile([128, 512], f32, name="s",
                                                   tag="s")
                                    nc.tensor.matmul(
                                        s[:, lead:512],
                                        kTt[:, jbl * 128:(jbl + 1) * 128],
                                        qT[tc4 // 2][h][:, (tc4 % 2) * 512
                                                        + lead:
                                                        (tc4 % 2 + 1) * 512],
                                        start=True, stop=True)
                                    pt = ptp.tile([128, 512], f16, name="pt",
                                                  tag="pt")
                                    bc = _bias_col(h, tcp, jb)
                                    nc.scalar.activation(
                                        pt[:, lead:512], s[:, lead:512],
                                        AF.Exp,
                                        bias=bias_sb[:, bc:bc + 1], scale=1.0)
                                    if o + 128 > glo:
                                        k = lead // 128
                                        w = min(o + 128, glo + 512) - glo
                                        nc.vector.tensor_tensor(
                                            pt[:, lead:w], pt[:, lead:w],
                                            mask_sb[:, k * 512 + lead:
                                                    k * 512 + w],
                                            ALU.min)
                                    # defer PV one block so the next S is in
                                    # the PE queue before PV blocks on exp
                                    pending.append((jb, tc4, pt, lead))
                                    if len(pending) > 3:
                                        emit_pv(*pending.pop(0))
                                    yield
                                    tick()
                                if jb == 4 * (2 * tcp) + 3:
                                    while pending:
                                        emit_pv(*pending.pop(0))
                                    deferred.append(
                                        [2, norm_recip(tcp, 0, y_acc,
                                                       hp, sub)])
                                    yield
                                    tick()
                            while pending:
                                emit_pv(*pending.pop(0))
                            deferred.append(
                                [2, norm_recip(tcp, 1, y_acc, hp, sub)])
                            yield
                            tick()

                    def emit_proj(tcp, hp, ypool, evac='dve', tag="pp"):
                        for cc in range(8):
                            for tcl in range(2):
                                tc4 = 2 * tcp + tcl
                                ps = ypool.tile([128, 512], f32, name="pp",
                                                tag=tag)
                                nc.tensor.matmul(
                                    ps[:],
                                    wpt[hp][:, cc * 128:(cc + 1) * 128],
                                    yT[tcp][hp][:, tcl * 512:(tcl + 1) * 512],
                                    start=True, stop=True)
                                ob = obp.tile([128, 512], f16, name="ob",
                                              tag="ob")
                                if evac == 'act':
                                    nc.scalar.copy(ob[:], ps[:])
                                elif evac == 'mix' and cc % 2 == 0:
                                    nc.scalar.copy(ob[:], ps[:])
                                else:
                                    nc.vector.tensor_copy(ob[:], ps[:])
                                nc.sync.dma_start(
                                    out_d[hp].ap()[cc * 128:(cc + 1) * 128,
                                                   tc4 * 512:(tc4 + 1) * 512],
                                    ob[:])
                                yield

                    def drain(g):
                        for _ in g:
                            pass

                    def chain(*gens):
                        for g in gens:
                            yield from g

                    def interleave(ga, gb, ratio, drain_b=True):
                        alive_a = alive_b = True
                        while alive_a or (alive_b and drain_b):
                            for _ in range(ratio):
                                if alive_a:
                                    try:
                                        next(ga)
                                    except StopIteration:
                                        alive_a = False
                            if alive_b:
                                try:
                                    next(gb)
                                except StopIteration:
                                    alive_b = False

                    def pad(n):
                        for _ in range(n):
                            yield

                    def merge2(ga, gb):
                        alive_a = alive_b = True
                        while alive_a or alive_b:
                            if alive_a:
                                try:
                                    next(ga)
                                except StopIteration:
                                    alive_a = False
                            if alive_b:
                                try:
                                    next(gb)
                                except StopIteration:
                                    alive_b = False
                            yield

                    with tc.tile_pool(name="pt", bufs=8) as ptp, \
                         tc.tile_pool(name="nrm", bufs=4) as nrmp, \
                         tc.tile_pool(name="ob", bufs=3) as obp, \
                         tc.tile_pool(name="s", bufs=2, space="PSUM") as sp, \
                         tc.tile_pool(name="y", bufs=4, space="PSUM") as yp:
                        deferred = []   # [countdown, norm-apply fn]
                        with tc.tile_pool(name="bld", bufs=2,
                                          space="PSUM") as bld:
                            emit_xdma(0)
                            drain(emit_qk(0, 0, bld))
                            emit_xdma(1)
                            for hp in range(NHP):
                                nc.gpsimd.dma_start(
                                    wpt[hp][:],
                                    wp_d.ap()[hp * 128:(hp + 1) * 128, :])
                            if phases == 'build':
                                drain(emit_v(0, bld))
                                drain(emit_qk(0, 1, bld))
                                drain(emit_qk(1, 0, bld))
                                drain(emit_v(1, bld))
                                drain(emit_qk(1, 1, bld))
                            else:
                                bcpool["p"] = bld
                                bcpool["tag"] = "bld"
                                interleave(
                                    emit_attn([(0, 0)], sp, yp, deferred),
                                    chain(emit_v(0, bld),
                                          emit_qk(1, 0, bld)), 1)
                                interleave(
                                    merge2(
                                        emit_attn([(1, 0)], sp, yp,
                                                  deferred),
                                        emit_attn([(0, 1)], sp, yp,
                                                  deferred)),
                                    chain(emit_v(1, bld),
                                          emit_qk(0, 1, bld),
                                          emit_qk(1, 1, bld)), 1)
                        if phases != 'build':
                            with tc.tile_pool(name="pp", bufs=2,
                                              space="PSUM") as ppp:
                                bcpool["p"] = ppp
                                bcpool["tag"] = "pp"
                                bg = chain(pad(6),
                                           emit_proj(0, 0, ppp),
                                           pad(10),
                                           emit_proj(1, 0, ppp),
                                           pad(2),
                                           emit_proj(0, 1, ppp))
                                interleave(
                                    merge2(
                                        emit_attn([(1, 1)], sp, yp,
                                                  deferred),
                                        emit_attn([(0, 2)], sp, yp,
                                                  deferred)),
                                    bg, 1, drain_b=False)
                                interleave(
                                    merge2(
                                        emit_attn([(1, 2)], sp, yp,
                                                  deferred),
                                        emit_attn([(0, 3)], sp, yp,
                                                  deferred)),
                                    bg, 1, drain_b=False)
                                interleave(
                                    emit_attn([(1, 3)], sp, yp, deferred),
                                    bg, 2, drain_b=False)
                                while deferred:
                                    deferred.pop(0)[1]()
                                interleave(
                                    emit_proj(1, 1, yp, tag="y"),
                                    bg, 1)

            if phases == 'build':
                                drain(emit_v(0, bld))
                                drain(emit_qk(0, 1, bld))
                                drain(emit_qk(1, 0, bld))
                                drain(emit_v(1, bld))
                                drain(emit_qk(1, 1, bld))
                            else:
                                def build_rest1():
                                    yield from emit_v(0, bld)
                                    yield from emit_qk(1, 0, bld)
                                interleave(
                                    emit_attn([(0, 0)], sp, yp, deferred),
                                    build_rest1(), 1)
                                interleave(
                                    emit_attn([(1, 0)], sp, yp, deferred),
                                    emit_v(1, bld), 3)
                                def build_rest2():
                                    yield from emit_qk(0, 1, bld)
                                    yield from emit_qk(1, 1, bld)
                                interleave(
                                    emit_attn([(0, 1), (1, 1)], sp, yp,
                                              deferred),
                                    build_rest2(), 4)
                        if phases != 'build':
                            with tc.tile_pool(name="pp", bufs=2,
                                              space="PSUM") as ppp:
                                bg = chain(emit_proj(0, 0, ppp, 'mix'),
                                           emit_proj(1, 0, ppp, 'mix'),
                                           pad(6),
                                           emit_proj(0, 1, ppp, 'mix'))
                                interleave(
                                    emit_attn([(0, 2), (1, 2)], sp, yp,
                                              deferred),
                                    bg, 2, drain_b=False)
                                interleave(
                                    emit_attn([(0, 3), (1, 3)], sp, yp,
                                              deferred),
                                    bg, 2, drain_b=False)
                                while deferred:
                                    deferred.pop(0)[1]()
                                interleave(
                                    emit_proj(1, 1, yp, tag="y"),
                                    bg, 1)

            if phases == 'build':
            if phases == 'build':
                zt = cp.tile([128, 512], f16, tag="zt")
                nc.vector.memset(zt[:], 0.0)
                for p in range(NHP):
                    for cc in range(8):
                        for tc4 in range(NTC):
                            nc.sync.dma_start(
                                out_d[p].ap()[cc * 128:(cc + 1) * 128,
                                              tc4 * 512:(tc4 + 1) * 512],
                                zt[:])
            if nrep > 1:
                with tc.For_i(0, nrep, 1):
                    body()
            else:
                body()
    nc.compile()
    return nc


def shard_inputs(x, W_attn, b_attn, W_proj, b_proj):
    x = np.asarray(x, np.float32)
    W_attn = np.asarray(W_attn, np.float32)
    b_attn = np.asarray(b_attn, np.float32)
    W_proj = np.asarray(W_proj, np.float32)
    in_maps = []
    for core in range(NCORES):
        b = core // CPB
        hg = core % CPB
        heads = list(range(hg * HPC, (hg + 1) * HPC))
        rows, brows = [], []
        for typ in range(3):
            for h in heads:
                r0 = typ * C + h * DH
                w = W_attn[r0:r0 + DH]
                bb = b_attn[r0:r0 + DH]
                if typ == 0:
                    w = w * INV_SQRT_D
                    bb = bb * INV_SQRT_D
                rows.append(w)
                brows.append(bb)
        wfull = np.ascontiguousarray(np.concatenate(rows, 0).T)  # [C, 768]
        wqkT = np.ascontiguousarray(wfull[:, :2 * D_LOC]).astype(np.float16)
        wvT = np.ascontiguousarray(wfull[:, 2 * D_LOC:]).astype(np.float16)
        bqkv = np.concatenate(brows).astype(np.float32)   # [768]
        # bq6 [128, 6]: col typ*2+hp = bias rows for that (typ, head-pair)
        bq6 = np.zeros((128, 6), np.float32)
        for typ in range(2):
            for hp in range(NHP):
                bq6[:, typ * 2 + hp] = bqkv[typ * D_LOC + hp * 128:
                                            typ * D_LOC + (hp + 1) * 128]
        bvrow = bqkv[2 * D_LOC:3 * D_LOC][None, :].astype(np.float16)
        cols = np.concatenate([np.arange(h * DH, (h + 1) * DH) for h in heads])
        wpT = np.ascontiguousarray(W_proj[:, cols].T).astype(np.float16)
        tt = np.arange(HT, dtype=np.float32)
        aug = np.zeros((97, HT), np.float16)
        for hl, h in enumerate(heads):
            aug[32 * hl] = (-SLOPES[h] * tt).astype(np.float16)
        # exp-bias table [128, 24*HPC]: slope*(jb*128 + jp - tcp*1024) - M
        jp = np.arange(128, dtype=np.float32)
        bias = np.zeros((128, 24 * HPC), np.float32)
        for hl, h in enumerate(heads):
            for tcp in range(2):
                for jb in range(8 * (tcp + 1)):
                    bias[:, _bias_col(hl, tcp, jb)] = (
                        SLOPES[h] * (jb * 128 + jp - tcp * HT) - M_MARGIN)
        tt512 = np.arange(512)[None, :]
        jl = np.arange(128)[:, None]
        maskcap = np.zeros((128, 4 * 512), np.float16)
        for k in range(4):
            maskcap[:, k * 512:(k + 1) * 512] = np.where(
                tt512 >= k * 128 + jl, np.float16(65504.0), np.float16(0.0))
        in_maps.append({
            "xT": np.ascontiguousarray(x[b].T).astype(np.float16),
            "wqkT": wqkT, "wvT": wvT, "bq6": bq6, "bvrow": bvrow, "wpT": wpT,
            "aug": aug, "biascols": bias, "maskcap": maskcap,
            "onesrow": np.ones((1, HT), np.float16),
        })
    return in_maps


def unshard(results, b_proj):
    y = np.zeros((B, T, C), np.float32)
    for core in range(NCORES):
        b = core // CPB
        for p in range(NHP):
            y[b] += results[core][f"out{p}"].T.astype(np.float32)
    y += np.asarray(b_proj, np.float32)[None, None, :]
    return y


_BUILD_CACHE = {}


def _built(nrep: int = 1):
    if nrep not in _BUILD_CACHE:
        _BUILD_CACHE[nrep] = build(nrep)
    return _BUILD_CACHE[nrep]


def kernel(**inputs) -> np.ndarray:
    in_maps = shard_inputs(inputs["x"], inputs["W_attn"], inputs["b_attn"],
                           inputs["W_proj"], inputs["b_proj"])
    nc = _built(1)
    res = run_bass_kernel_spmd(nc, in_maps, core_ids=list(range(NCORES)))
    return unshard(res.results, inputs["b_proj"])
